# revision 1
# baseline (speedup 1.0000x reference)
# Min-plus (tropical) matmul kernel for Trainium2, 8 NeuronCores.
#
#   y[n,o] = min_i (x[n,i] + w[o,i]) + bias[o]
#
# Strategy: the elementwise min cannot use the PE array, and an exact
# elementwise evaluation on DVE costs ~550us/core. Instead we evaluate a
# temperature-cascaded softmin whose heavy lifting is ordinary matmuls:
#
#   S_t[n,o] = sum_i exp(-t(x[n,i]-a_n-m)) * exp(-t(w[o,i]-b_o-m))
#            = exp(2tm) * sum_i exp(-t(v_i - a_n - b_o))          (v = x+w)
#   P_t[n,o] = same sums weighted by (x-a) and (w+bias)
#   moment estimate: P/S + a        (>= y, first-moment softmin average)
#   lse    estimate: -ln(S)/t + a + b + 2m + bias   (<= y)
#   blend = 0.5*moment + 0.5*lse    (errors have opposite signs)
#
# Temperature is limited by fp32 range: entries exp(-t(x-a-m)) must keep the
# argmin term normal. With per-row/col shifts a_n, b_o, a per-side raise m
# (2tm <= 78) and entry cleaning at tau=1e-34, t1 = (76+39)/Rw is certified
# for every element (Rw = max row range of w >= any Delta = y-a-b). Higher
# levels t_k = 2^k t1 are used per element only where the level-1 estimate
# certifies Delta_hat small enough (validity is a compile-time-free compare).
#
# Cleaning (A = (A>=tau)*A) is load-bearing: it prevents "junk mass" where a
# value-weighted entry survives fp32 while its plain entry flushed, which
# would corrupt P/S.

import numpy as np
from contextlib import ExitStack

import concourse.bass as bass
import concourse.mybir as mybir
import concourse.tile as tile
from concourse import bacc
from concourse import bass_utils
from concourse.masks import make_identity

FP = mybir.dt.float32
FPR = mybir.dt.float32r
AF = mybir.ActivationFunctionType
OP = mybir.AluOpType

N_CORES = 8
DIN = 128
DOUT = 128
TAU = 1e-36     # entry cleaning threshold
KM = 78.0       # certified nats (with margin) for argmin entry survival (< ln(1/TAU)=82.9)
RB = 18.5       # per-side raise budget: t*m = RB; keeps S <= 128*e^37 within ACT Ln domain


def softmin_cfg(weight: np.ndarray):
    """Host-side scalar tuning derived from the small weight tensor only."""
    Rw = float(np.ptp(weight.astype(np.float64), axis=1).max())
    Rw = max(Rw, 1e-3)
    t1 = min((KM + RB) / Rw, 4000.0)
    ts = [t1, 2.0 * t1]
    ms = [RB / t for t in ts]
    slack = float(np.log(128.0) / (2.0 * t1))
    thetas = [KM / t + m - slack for t, m in zip(ts, ms)]  # validity: H <= theta_k
    return ts, ms, thetas


def _bcast_mid(ap2d: bass.AP, mid: int) -> bass.AP:
    """[P, F] -> [P, mid, F] with stride-0 middle dim."""
    return bass.AP(tensor=ap2d.tensor, offset=ap2d.offset,
                   ap=[ap2d.ap[0], [0, mid], ap2d.ap[1]])


def minplus_body(tc, outs, ins, cfg):
    """Tile kernel body. ins/outs are DRAM APs:
      ins: x [SH,128], w [128,128], bias [128]; outs: y [SH,128].
    cfg: dict(ts, ms, thetas, shard_rows, bb_stage=dram AP [128])"""
    nc = tc.nc
    ts, ms, thetas = cfg["ts"], cfg["ms"], cfg["thetas"]
    SH = cfg["shard_rows"]
    BLK = SH // 128
    WAVE = min(4, BLK)
    NWAVES = BLK // WAVE
    assert BLK * 128 == SH and NWAVES * WAVE == BLK

    xd = ins["x"].rearrange("(blk p) i -> p blk i", p=128)
    yd = outs["y"].rearrange("(blk p) o -> p blk o", p=128)
    wd, biasd = ins["w"], ins["bias"]
    bb_stage = cfg["bb_stage"]

    with ExitStack() as ctx:
        big = ctx.enter_context(tc.tile_pool(name="big", bufs=1))
        ring = ctx.enter_context(tc.tile_pool(name="ring", bufs=2))
        psum = ctx.enter_context(tc.tile_pool(name="psum", bufs=2, space="PSUM"))

        # ---- loads ----
        X = big.tile([128, BLK, DIN], FP, tag="X_F2")
        for hh in range(2):
            hs = slice(hh * (BLK // 2), (hh + 1) * (BLK // 2))
            nc.sync.dma_start(out=X[:, hs, :], in_=xd[:, hs, :])
        Wsb = big.tile([128, DIN], FP)
        nc.sync.dma_start(out=Wsb, in_=wd)
        bias_col = big.tile([128, 1], FP)
        nc.sync.dma_start(out=bias_col, in_=biasd.rearrange("(o u) -> o u", u=1))

        ident = big.tile([128, 128], FP)
        make_identity(nc, ident)
        b39 = big.tile([128, 1], FP)
        nc.vector.memset(b39, RB)
        zb = big.tile([128, 1], FP)
        nc.vector.memset(zb, 0.0)

        # ---- w-side prep ----
        bcol = big.tile([128, 1], FP)
        nc.vector.tensor_reduce(bcol, Wsb, axis=mybir.AxisListType.X, op=OP.min)
        dw = big.tile([128, DIN], FP)
        nc.vector.tensor_scalar(out=dw, in0=Wsb, scalar1=bcol, scalar2=None,
                                op0=OP.subtract)
        # bb = b + bias, replicated across partitions via DRAM round-trip
        bb_col = big.tile([128, 1], FP)
        nc.vector.tensor_tensor(out=bb_col, in0=bcol, in1=bias_col, op=OP.add)
        nc.sync.dma_start(out=bb_stage.rearrange("(o u) -> o u", u=1), in_=bb_col)
        bbrep = big.tile([128, 128], FP)
        bb_b = bass.AP(tensor=bb_stage.tensor, offset=bb_stage.offset,
                       ap=[[0, 128], bb_stage.ap[0]])
        nc.sync.dma_start(out=bbrep, in_=bb_b)

        # ---- x-side shared prep ----
        a = big.tile([128, BLK], FP)
        for hh in range(2):
            hs = slice(hh * (BLK // 2), (hh + 1) * (BLK // 2))
            nc.vector.tensor_reduce(a[:, hs], X[:, hs, :],
                                    axis=mybir.AxisListType.X, op=OP.min)
        d = big.tile([128, BLK, DIN], FP)
        a_b = bass.AP(tensor=a.tensor, offset=a.offset,
                      ap=[a.ap[0], a.ap[1], [0, DIN]])
        # d = 0.5*X - 0.5*a: the 0.5 pre-scales the moment value weights so
        # the final blend is a single fused op; exp args use scale=-2t.
        a2 = big.tile([128, BLK], FP)
        nc.vector.tensor_scalar(out=a2, in0=a, scalar1=0.5, scalar2=None,
                                op0=OP.mult)
        a2_b = bass.AP(tensor=a2.tensor, offset=a2.offset,
                       ap=[a2.ap[0], a2.ap[1], [0, DIN]])
        for hh in range(2):
            hs = slice(hh * (BLK // 2), (hh + 1) * (BLK // 2))
            a2_bh = bass.AP(tensor=a2.tensor, offset=a2.offset + hh * (BLK // 2),
                            ap=[a2.ap[0], [a2.ap[1][0], BLK // 2], [0, DIN]])
            nc.vector.scalar_tensor_tensor(out=d[:, hs, :], in0=X[:, hs, :],
                                           scalar=0.5, in1=a2_bh,
                                           op0=OP.mult, op1=OP.subtract)
        # dT = transpose(d) once; per-level exp/clean/mult happen in [i, n]
        dT = big.tile([128, BLK, 128], FP)
        TCH = min(8, BLK)
        for h in range(BLK // TCH):
            pch = psum.tile([128, TCH, 128], FP, tag="p8")
            for j in range(TCH):
                nc.tensor.transpose(pch[:, j, :], d[:, h * TCH + j, :], ident)
            cs = slice(h * TCH, (h + 1) * TCH)
            nc.vector.tensor_copy(dT[:, cs, :], pch)

        F1 = big.tile([128, BLK, DOUT], FP)
        F2 = big.tile([128, BLK, DOUT], FP, tag="X_F2")
        mask = big.tile([128, BLK, DOUT], mybir.dt.uint8)
        bbth = big.tile([128, 128], FP)

        for lvl, (t, m, theta) in enumerate(zip(ts, ms, thetas)):
            Fdst = F1 if lvl == 0 else F2

            # ---- w-side per level ----
            Bex = big.tile([128, DIN], FP, tag="Bex")
            nc.scalar.activation(Bex, dw, AF.Exp, bias=b39, scale=-t)
            nc.vector.scalar_tensor_tensor(out=Bex, in0=Bex, scalar=TAU,
                                           in1=Bex, op0=OP.is_ge, op1=OP.mult)
            # value weight: 0.5*w + kappa_o, kappa = bias + 0.5*b + m
            # (carries the lse-side per-o constant so Sigma p * value folds it)
            kap = big.tile([128, 1], FP, tag="kap")
            nc.vector.tensor_scalar(out=kap, in0=bcol, scalar1=0.5,
                                    scalar2=None, op0=OP.mult)
            nc.vector.tensor_tensor(out=kap, in0=kap, in1=bias_col, op=OP.add)
            nc.vector.tensor_scalar(out=kap, in0=kap, scalar1=float(m),
                                    scalar2=None, op0=OP.add)
            Bv = big.tile([128, DIN], FP, tag="Bv")
            nc.vector.tensor_scalar(out=Bv, in0=Wsb, scalar1=0.5, scalar2=kap,
                                    op0=OP.mult, op1=OP.add)
            nc.vector.tensor_tensor(out=Bv, in0=Bv, in1=Bex, op=OP.mult)
            # transpose w-side into RHS = [BexT | BvT | 0 | BexT]
            pT = psum.tile([128, 2, 128], FP, tag="p8")
            nc.tensor.transpose(pT[:, 0, :], Bex, ident)
            nc.tensor.transpose(pT[:, 1, :], Bv, ident)
            RHS = big.tile([128, 4 * DOUT], FPR, tag="RHS")
            nc.vector.tensor_copy(RHS[:, 0:128], pT[:, 0, :])
            nc.vector.tensor_copy(RHS[:, 384:512], pT[:, 0, :])
            nc.vector.tensor_copy(RHS[:, 128:256], pT[:, 1, :])
            nc.vector.memset(RHS[:, 256:384].bitcast(mybir.dt.int32), 0)

            # ---- x-side per level (transposed layout), half-tiles so the
            # exp/clean/mult chain pipelines across halves ----
            At = ring.tile([128, BLK, 128], FPR, tag="At")
            A1t = ring.tile([128, BLK, 128], FPR, tag="A1t")
            HB = BLK // 4
            for hh in range(4):
                hs = slice(hh * HB, (hh + 1) * HB)
                nc.scalar.activation(At[:, hs, :], dT[:, hs, :], AF.Exp,
                                     bias=b39, scale=-2.0 * t)
                nc.vector.scalar_tensor_tensor(out=At[:, hs, :],
                                               in0=At[:, hs, :], scalar=TAU,
                                               in1=At[:, hs, :],
                                               op0=OP.is_ge, op1=OP.mult)
                nc.gpsimd.tensor_tensor(out=A1t[:, hs, :], in0=dT[:, hs, :],
                                        in1=At[:, hs, :], op=OP.mult)

            # ---- matmul waves: M = P/S and sqrt(S) per wave (psum-direct);
            # uncertified elements may yield Inf/NaN here and are masked out
            # at the select step.
            Qf = big.tile([128, BLK, 128], FP, tag="Qf")
            Mf = big.tile([128, BLK, 128], FP, tag="Mf")
            for wv in range(NWAVES):
                SP = psum.tile([128, WAVE, 512], FP, tag="p8")
                for j in range(WAVE):
                    blk = wv * WAVE + j
                    nc.tensor.matmul(SP[:, j, 0:256], lhsT=At[:, blk, :],
                                     rhs=RHS[:, 0:256], start=True, stop=False)
                    nc.tensor.matmul(SP[:, j, 0:256], lhsT=A1t[:, blk, :],
                                     rhs=RHS[:, 256:512], start=False, stop=True)
                Sv = SP[:, :, 0:128]
                Pv = SP[:, :, 128:256]
                ws = slice(wv * WAVE, (wv + 1) * WAVE)
                r = big.tile([128, WAVE, 128], FP, tag="r")
                nc.vector.reciprocal_approx_fast(out=r, in_=Sv)
                nc.vector.tensor_tensor(out=Mf[:, ws, :], in0=Pv, in1=r,
                                        op=OP.mult)
                nc.scalar.activation(Qf[:, ws, :], Sv, AF.Sqrt, bias=zb,
                                     scale=1.0)

            # ---- batched lse + blend for the level ----
            # ACT Ln is only accurate on [2^-64, 2^64]; ln(S) via 2*ln(sqrt(S))
            # compresses S's certified span [e^-78, e^42] into the domain.
            Lf = big.tile([128, BLK, 128], FP, tag="Lf")
            for hh in range(2):
                hs = slice(hh * (BLK // 2), (hh + 1) * (BLK // 2))
                nc.scalar.activation(Lf[:, hs, :], Qf[:, hs, :], AF.Ln,
                                     bias=zb, scale=1.0)
                # F = M + (-1/t)*ln(sqrt S)  (constants folded into M weights)
                nc.vector.scalar_tensor_tensor(out=Fdst[:, hs, :],
                                               in0=Lf[:, hs, :],
                                               scalar=-1.0 / t,
                                               in1=Mf[:, hs, :],
                                               op0=OP.mult, op1=OP.add)

            if lvl >= 1:
                # select Fdst into F1 where certified valid at this level:
                # F1 - bb <= theta  <=>  F1 <= bb + theta
                nc.vector.tensor_scalar(out=bbth, in0=bbrep, scalar1=theta,
                                        scalar2=None, op0=OP.add)
                for hh in range(2):
                    hs = slice(hh * (BLK // 2), (hh + 1) * (BLK // 2))
                    nc.vector.tensor_tensor(out=mask[:, hs, :], in0=F1[:, hs, :],
                                            in1=_bcast_mid(bbth, BLK // 2),
                                            op=OP.is_le)
                    nc.vector.copy_predicated(F1[:, hs, :], mask[:, hs, :],
                                              Fdst[:, hs, :])

        # ---- final: y = F1 + a, store ----
        Y = big.tile([128, BLK, DOUT], FP, tag="X_F2")
        for hh in range(2):
            hs = slice(hh * (BLK // 2), (hh + 1) * (BLK // 2))
            a_bh = bass.AP(tensor=a.tensor, offset=a.offset + hh * (BLK // 2),
                           ap=[a.ap[0], [a.ap[1][0], BLK // 2], [0, DIN]])
            nc.vector.tensor_tensor(out=Y[:, hs, :], in0=F1[:, hs, :],
                                    in1=a_bh, op=OP.add)
        nc.sync.dma_start(out=yd, in_=Y)


def build_nc(shard_rows: int, weight: np.ndarray):
    ts, ms, thetas = softmin_cfg(weight)
    nc = bacc.Bacc()
    x_d = nc.dram_tensor("x", [shard_rows, DIN], FP, kind="ExternalInput")
    w_d = nc.dram_tensor("w", [DOUT, DIN], FP, kind="ExternalInput")
    bias_d = nc.dram_tensor("bias", [DOUT], FP, kind="ExternalInput")
    y_d = nc.dram_tensor("y", [shard_rows, DOUT], FP, kind="ExternalOutput")
    bb_stage = nc.dram_tensor("bb_stage", [DOUT], FP)
    cfg = dict(ts=ts, ms=ms, thetas=thetas, shard_rows=shard_rows,
               bb_stage=bb_stage[:])
    with tile.TileContext(nc) as tc:
        minplus_body(tc, {"y": y_d[:]}, {"x": x_d[:], "w": w_d[:], "bias": bias_d[:]}, cfg)
    nc.compile()
    return nc


def kernel(x: np.ndarray, weight: np.ndarray, bias: np.ndarray) -> np.ndarray:
    prefix = x.shape[:-1]
    x2 = np.ascontiguousarray(x, dtype=np.float32).reshape(-1, DIN)
    n = x2.shape[0]
    # rows are independent; zero-pad to a full 8-core, 128-row-block multiple
    step = N_CORES * 128
    n_pad = (n + step - 1) // step * step
    if n_pad != n:
        x2 = np.concatenate([x2, np.zeros((n_pad - n, DIN), np.float32)], 0)
    shard = n_pad // N_CORES
    w = np.ascontiguousarray(weight, dtype=np.float32)
    b = np.ascontiguousarray(bias, dtype=np.float32)

    nc = build_nc(shard, w)
    in_maps = [{"x": np.ascontiguousarray(x2[c * shard:(c + 1) * shard]),
                "w": w, "bias": b} for c in range(N_CORES)]
    res = bass_utils.run_bass_kernel_spmd(nc, in_maps, core_ids=list(range(N_CORES)))
    y = np.concatenate([res.results[c]["y"] for c in range(N_CORES)], axis=0)
    return y[:n].reshape(*prefix, DOUT)


if __name__ == "__main__":
    rng = np.random.default_rng(0)
    x = rng.standard_normal((16, 2048, 128)).astype(np.float32)
    w = rng.standard_normal((128, 128)).astype(np.float32)
    b = rng.standard_normal(128).astype(np.float32)
    y = kernel(x, w, b)
    ref = (x[..., None, :] + w[None, None, :, :]).min(-1) + b
    err = np.abs(y - ref)
    print("max err:", err.max(), "rel absmax:", err.max() / np.abs(ref).max())

